# revision 29
# baseline (speedup 1.0000x reference)
"""Causal self-attention Trainium2 kernel (B=2, T=4096, E=768, H=12, D=64).

Sharding: 8 cores = 2 batches x 4 head-groups (3 heads each). Each core:
  - computes q/k in transposed layout [d, t] and v in natural layout [t, d]
    for its 3 heads (fp32r matmuls, PE transposes for x^T),
  - causal attention in S^T layout ([key, query] tiles) so softmax
    normalization needs no P transposes: exp on ACT, denominator via an
    extra ones-column appended to v (PV matmul row 64 = sum of exp),
  - normalizes via reciprocal + PE broadcast, out-projects with its wo
    row-slice producing a partial y [4096, 768].
Host sums the 4 partials per batch and adds bo.
"""

import os
import sys

sys.path.insert(0, "/opt/trn_rl_repo")

import numpy as np

try:  # persistent jit cache: skips the ~10min neuronxcc compile on re-runs
    import jax

    jax.config.update("jax_compilation_cache_dir", "/tmp/jax_neff_cache")
    jax.config.update("jax_persistent_cache_min_compile_time_secs", 10)
    jax.config.update("jax_persistent_cache_min_entry_size_bytes", 0)
except Exception:
    pass

import concourse.bass as bass
import concourse.mybir as mybir
import concourse.tile as tile
from concourse import bacc
from concourse.bass_utils import run_bass_kernel_spmd

F32 = mybir.dt.float32
F32R = mybir.dt.float32r

B, T, E, H = 2, 4096, 768, 12
D = E // H            # 64
HL = 3                # heads per core
CH = HL * D           # 192 channels per core
SB = 512              # query superblock
KB = 128              # key block
NEB = E // 128        # 6 embed tiles
SCALE = 1.0 / np.sqrt(D)

USE_F32R = os.environ.get("ATTN_NO_F32R", "") == ""


def _mm(ap):
    return ap.bitcast(F32R) if USE_F32R else ap


def build_nc(t_len=T):
    assert t_len % SB == 0
    nsb = t_len // SB       # superblocks
    ntb = t_len // KB       # 128-blocks

    nc = bacc.Bacc("TRN2", target_bir_lowering=False, debug=False, num_devices=8)

    xb = nc.dram_tensor("xb", [t_len, E], F32, kind="ExternalInput")
    wqk = nc.dram_tensor("wqk", [E, 2 * CH], F32, kind="ExternalInput")
    wvp = nc.dram_tensor("wvp", [E, 256], F32, kind="ExternalInput")
    wo = nc.dram_tensor("wo", [CH, E], F32, kind="ExternalInput")
    bqk = nc.dram_tensor("bqk", [HL, 2, D], F32, kind="ExternalInput")
    bv = nc.dram_tensor("bv", [CH + HL], F32, kind="ExternalInput")
    cst = nc.dram_tensor("cst", [128, 192], F32, kind="ExternalInput")
    y = nc.dram_tensor("y", [t_len, E], F32, kind="ExternalOutput")

    xb, wqk, wvp, wo, bqk, bv, cst, y = (
        t.ap() for t in (xb, wqk, wvp, wo, bqk, bv, cst, y)
    )

    with tile.TileContext(nc) as tc:
        import contextlib

        ctx = contextlib.ExitStack()
        with ctx:
            ctx.enter_context(
                nc.allow_low_precision(reason="fp32r rounding of matmul operands")
            )
            const = ctx.enter_context(tc.tile_pool(name="const", bufs=1))
            persist = ctx.enter_context(tc.tile_pool(name="persist", bufs=1))
            xpool = ctx.enter_context(tc.tile_pool(name="xpool", bufs=2))
            xtpool = ctx.enter_context(tc.tile_pool(name="xtpool", bufs=2))
            qspool = ctx.enter_context(tc.tile_pool(name="qspool", bufs=4))
            ospool = ctx.enter_context(tc.tile_pool(name="ospool", bufs=2))
            ptpool = ctx.enter_context(tc.tile_pool(name="ptpool", bufs=3))
            rpool = ctx.enter_context(tc.tile_pool(name="rpool", bufs=2))
            ypool = ctx.enter_context(tc.tile_pool(name="ypool", bufs=2))
            psA = ctx.enter_context(tc.tile_pool(name="psA", bufs=2, space="PSUM"))
            psS = ctx.enter_context(tc.tile_pool(name="psS", bufs=2, space="PSUM"))
            psO = ctx.enter_context(tc.tile_pool(name="psO", bufs=2, space="PSUM"))

            # ---- constants / weights in SBUF ----
            # identity and the broadcast-ones row come in via DMA: walrus
            # requires f32r-typed producers for matmul operands and its ISA
            # rejects f32r memsets
            ident = const.tile([128, 128], F32)
            nc.sync.dma_start(out=_mm(ident), in_=_mm(cst[:, 0:128]))
            ones65 = const.tile([65, D], F32)
            nc.sync.dma_start(
                out=_mm(ones65[64:65, :]), in_=_mm(cst[64:65, 128 : 128 + D])
            )

            wqk_sb = const.tile([128, NEB, 2 * CH], F32)
            nc.sync.dma_start(
                out=_mm(wqk_sb), in_=_mm(wqk).rearrange("(n p) m -> p n m", p=128)
            )
            wv_sb = const.tile([128, NEB, 256], F32)
            nc.sync.dma_start(
                out=_mm(wv_sb), in_=_mm(wvp).rearrange("(n p) m -> p n m", p=128)
            )
            wo01_sb = const.tile([128, E], F32)
            nc.sync.dma_start(out=_mm(wo01_sb), in_=_mm(wo[0 : 2 * D, :]))
            wo2_sb = const.tile([D, E], F32)
            nc.sync.dma_start(out=_mm(wo2_sb), in_=_mm(wo[2 * D : CH, :]))
            bqk_sb = const.tile([D, HL, 2], F32)
            nc.sync.dma_start(out=bqk_sb, in_=bqk.rearrange("h q p -> p h q"))
            # k-bias copy living at partitions 64..127 (k rows of the packed
            # qk psum) so the staging add is partition-aligned
            bk64_sb = const.tile([128, HL], F32)
            nc.sync.dma_start(
                out=bk64_sb[D : 2 * D, :], in_=bqk[:, 1, :].rearrange("h p -> p h")
            )
            bv_bc = const.tile([128, CH + HL], F32)
            nc.sync.dma_start(
                out=bv_bc,
                in_=bass.AP(
                    tensor=bv.tensor, offset=bv.offset, ap=[[0, 128]] + list(bv.ap)
                ),
            )

            # persistent activations
            kT = [persist.tile([D, t_len], F32, name=f"kT{h}") for h in range(HL)]
            v_sb = persist.tile([128, ntb, HL, D + 1], F32)

            def phase3(cp, oS):
                # out-projection for t-blocks of superblock cp
                for tb in range(4):
                    tg = cp * 4 + tb
                    y_sb = ypool.tile([128, E], F32, tag="y_sb", name="y_sb")
                    oS01p, oS2p = oS
                    for half in range(2):
                        ps_y = psO.tile([128, 384], F32, tag="psO", name="ps_y")
                        nc.tensor.matmul(
                            ps_y,
                            lhsT=_mm(oS01p[:, tb * KB : (tb + 1) * KB]),
                            rhs=_mm(wo01_sb[:, half * 384 : (half + 1) * 384]),
                            start=True,
                            stop=False,
                        )
                        nc.tensor.matmul(
                            ps_y,
                            lhsT=_mm(oS2p[:, tb * KB : (tb + 1) * KB]),
                            rhs=_mm(wo2_sb[:, half * 384 : (half + 1) * 384]),
                            start=False,
                            stop=True,
                        )
                        nc.vector.tensor_copy(
                            out=y_sb[:, half * 384 : (half + 1) * 384], in_=ps_y
                        )
                    nc.sync.dma_start(
                        out=y[tg * KB : (tg + 1) * KB, :], in_=y_sb
                    )

            for c in range(nsb):
                # ======== phase 1: x^T, q^T/k^T, v for tokens [c*SB, (c+1)*SB)
                x_nat = xpool.tile([128, 4, E], F32, tag="x")
                for tb in range(4):
                    t0 = c * SB + tb * KB
                    nc.sync.dma_start(
                        out=_mm(x_nat[:, tb, :]), in_=_mm(xb[t0 : t0 + KB, :])
                    )
                xT = xtpool.tile([128, NEB, SB], F32, tag="xT")
                for eb in range(NEB):
                    ps_t = psA.tile([128, SB], F32, tag="psA", name="ps_t")
                    for tb in range(4):
                        nc.tensor.transpose(
                            _mm(ps_t[:, tb * 128 : (tb + 1) * 128]),
                            _mm(x_nat[:, tb, eb * 128 : (eb + 1) * 128]),
                            _mm(ident),
                        )
                    nc.vector.tensor_copy(out=_mm(xT[:, eb, :]), in_=ps_t)
                qS = []
                for h in range(HL):
                    ps_qk = psA.tile([128, SB], F32, tag="psA", name="ps_qk")
                    for eb in range(NEB):
                        nc.tensor.matmul(
                            ps_qk,
                            lhsT=_mm(wqk_sb[:, eb, h * 128 : (h + 1) * 128]),
                            rhs=_mm(xT[:, eb, :]),
                            start=(eb == 0),
                            stop=(eb == NEB - 1),
                        )
                    q_h = qspool.tile([D, SB], F32, tag="qS", name="q_h")
                    nc.vector.tensor_scalar_add(
                        out=_mm(q_h), in0=ps_qk[0:D, :], scalar1=bqk_sb[:, h, 0:1]
                    )
                    qS.append(q_h)
                    # k rows live at psum partitions 64..127. Lane engines
                    # cannot shift partitions, so stage at the same partitions
                    # (adding bias) and let an SBUF->SBUF DMA move them to
                    # partition base 0 in kT.
                    kst = qspool.tile([128, SB], F32, tag="kst", name="kst", bufs=2)
                    nc.vector.tensor_scalar_add(
                        out=_mm(kst[D : 2 * D, :]),
                        in0=ps_qk[D : 2 * D, :],
                        scalar1=bk64_sb[D : 2 * D, h : h + 1],
                    )
                    nc.sync.dma_start(
                        out=_mm(kT[h][:, c * SB : (c + 1) * SB]),
                        in_=_mm(kst[D : 2 * D, :]),
                    )
                for tb in range(4):
                    ps_v = psA.tile([128, 256], F32, tag="psA", name="ps_v")
                    for eb in range(NEB):
                        nc.tensor.matmul(
                            ps_v,
                            lhsT=_mm(xT[:, eb, tb * 128 : (tb + 1) * 128]),
                            rhs=_mm(wv_sb[:, eb, :]),
                            start=(eb == 0),
                            stop=(eb == NEB - 1),
                        )
                    nc.vector.tensor_add(
                        out=_mm(v_sb[:, c * 4 + tb, :, 0:D]),
                        in0=ps_v[:, 0:CH].rearrange("p (h d) -> p h d", h=HL),
                        in1=bv_bc[:, 0:CH].rearrange("p (h d) -> p h d", h=HL),
                    )
                    # ones column of v_aug: psum cols CH..CH+2 are x @ 0 = 0,
                    # plus the ones carried in the padded bias
                    nc.vector.tensor_add(
                        out=_mm(v_sb[:, c * 4 + tb, :, D : D + 1]),
                        in0=ps_v[:, CH : CH + HL].rearrange(
                            "p (h o) -> p h o", o=1
                        ),
                        in1=bv_bc[:, CH : CH + HL].rearrange(
                            "p (h o) -> p h o", o=1
                        ),
                    )

                # phase 3 of previous superblock goes here: its inputs (oS)
                # are produced by a DVE chain that lags PE, so slot the
                # already-runnable phase-1 work of this chunk in front of it
                if c > 0:
                    phase3(c - 1, oS_prev)

                # ======== phase 2: attention superblock i=c, all local heads
                nj = 4 * c + 4
                oS01 = ospool.tile([128, SB], F32, tag="oS01", name="oS01")
                oS2 = ospool.tile([D, SB], F32, tag="oS2", name="oS2")
                oS_prev_local = (oS01, oS2)
                for h in range(HL):
                    ps_o = psO.tile([128, SB], F32, tag="psO", name="ps_o")
                    q_ap = _mm(qS[h])

                    def q0_of(j):
                        # causal slice: key block j only sees queries
                        # >= j*KB - c*SB; keep the moving dim >= 256 so
                        # fp32r stays at full rate
                        if j < 4 * c:
                            return 0
                        return min((j - 4 * c) * KB, SB - 256)

                    def pv_step(j, pt_ap):
                        q0 = q0_of(j)
                        nc.tensor.matmul(
                            ps_o[0 : D + 1, q0:],
                            lhsT=_mm(v_sb[:, j, h, :]),
                            rhs=_mm(pt_ap[:, q0:]),
                            start=(j == 0),
                            stop=(j == nj - 1),
                        )

                    # process key blocks in pairs sharing one PSUM tile and
                    # (where widths match) a single wide exp on ACT, with a
                    # 2-block-deep software pipeline so PE stays ahead of ACT
                    pend = []
                    for jp in range(nj // 2):
                        j0, j1 = 2 * jp, 2 * jp + 1
                        q00, q01 = q0_of(j0), q0_of(j1)
                        ps_s2 = psS.tile([128, 2, SB], F32, tag="psS", name="ps_s2")
                        pt2 = ptpool.tile([128, 2, SB], F32, tag="pt", name="pt2")
                        for half, (j, q0) in enumerate(((j0, q00), (j1, q01))):
                            nc.tensor.matmul(
                                ps_s2[:, half, q0:],
                                lhsT=_mm(kT[h][:, j * KB : (j + 1) * KB]),
                                rhs=q_ap[:, q0:],
                                start=True,
                                stop=True,
                            )
                        if q00 == q01:
                            nc.scalar.activation(
                                out=_mm(pt2[:, :, q00:]),
                                in_=ps_s2[:, :, q00:],
                                func=mybir.ActivationFunctionType.Exp,
                                scale=float(SCALE),
                            )
                        else:
                            for half, q0 in ((0, q00), (1, q01)):
                                nc.scalar.activation(
                                    out=_mm(pt2[:, half, q0:]),
                                    in_=ps_s2[:, half, q0:],
                                    func=mybir.ActivationFunctionType.Exp,
                                    scale=float(SCALE),
                                )
                        for half, (j, q0) in enumerate(((j0, q00), (j1, q01))):
                            if j >= 4 * c:
                                nc.gpsimd.affine_select(
                                    out=_mm(pt2[:, half, q0:]),
                                    in_=_mm(pt2[:, half, q0:]),
                                    compare_op=mybir.AluOpType.is_ge,
                                    fill=0.0,
                                    base=c * SB - j * KB + q0,
                                    pattern=[[1, SB - q0]],
                                    channel_multiplier=-1,
                                )
                        pend.append((j0, pt2[:, 0, :]))
                        pend.append((j1, pt2[:, 1, :]))
                        while len(pend) > 4:
                            pv_step(*pend.pop(0))
                    for jq in pend:
                        pv_step(*jq)

                    recip = rpool.tile([65, SB], F32, tag="recip", name="recip")
                    nc.vector.reciprocal(_mm(recip[64:65, :]), ps_o[D : D + 1, :])
                    ps_b = psS.tile([128, SB], F32, tag="psS", name="ps_b")
                    nc.tensor.matmul(
                        ps_b[0:D, :],
                        lhsT=_mm(ones65[64:65, :]),
                        rhs=_mm(recip[64:65, :]),
                        start=True,
                        stop=True,
                    )
                    # walrus: a DVE op may read only ONE non-scalar PSUM input,
                    # so stage the broadcast row in SBUF before the multiply
                    rb = rpool.tile([D, SB], F32, tag="rbcast", name="rb")
                    nc.vector.tensor_copy(out=rb, in_=ps_b[0:D, :])
                    if h == 0:
                        o_dst = oS01[0:D, :]
                    elif h == 2:
                        o_dst = oS2[:, :]
                    else:
                        o_dst = ospool.tile(
                            [D, SB], F32, tag="o1tmp", name="o1tmp"
                        )
                    nc.vector.tensor_mul(_mm(o_dst), ps_o[0:D, :], rb)
                    if h == 1:
                        # stack h1 under h0 (partitions 64:128) via DMA, the
                        # only engine that can shift partitions
                        nc.sync.dma_start(out=_mm(oS01[D : 2 * D, :]), in_=_mm(o_dst))
                oS_prev = oS_prev_local
            phase3(nsb - 1, oS_prev)
    nc.compile()
    return nc


def make_in_maps(x, wq, bq, wk, bk, wv, bv, wo, bo, t_len=T):
    x = np.asarray(x, np.float32)
    in_maps = []
    for c in range(8):
        b, g = divmod(c, 4)
        hs = slice(g * CH, (g + 1) * CH)
        wqk_c = np.empty((E, 2 * CH), np.float32)
        bqk_c = np.empty((HL, 2, D), np.float32)
        for hl in range(HL):
            h = g * HL + hl
            wqk_c[:, hl * 128 : hl * 128 + D] = wq[:, h * D : (h + 1) * D]
            wqk_c[:, hl * 128 + D : (hl + 1) * 128] = wk[:, h * D : (h + 1) * D]
            bqk_c[hl, 0] = bq[h * D : (h + 1) * D]
            bqk_c[hl, 1] = bk[h * D : (h + 1) * D]
        wv_c = np.zeros((E, 256), np.float32)
        wv_c[:, :CH] = wv[:, hs]
        bv_c = np.ones(CH + HL, np.float32)
        bv_c[:CH] = np.asarray(bv, np.float32)[hs]
        cst = np.concatenate(
            [np.eye(128, dtype=np.float32), np.ones((128, 64), np.float32)], axis=1
        )
        in_maps.append(
            {
                "xb": np.ascontiguousarray(x[b, :t_len]),
                "wqk": wqk_c,
                "wvp": wv_c,
                "wo": np.ascontiguousarray(np.asarray(wo, np.float32)[hs]),
                "bqk": bqk_c,
                "bv": bv_c,
                "cst": cst,
            }
        )
    return in_maps


_NC_CACHE = {}


def get_nc(t_len=T):
    if t_len not in _NC_CACHE:
        _NC_CACHE[t_len] = build_nc(t_len)
    return _NC_CACHE[t_len]


def _build_sharded_nodonate(nc, n_cores=8):
    """Mirror bass2jax.run_bass_via_pjrt's multi-core path, minus donation,
    returning (jitted_fn, in_names, out_names, out_avals). Without donation a
    call can be repeated on device-resident arrays for timing. Safe here: the
    kernel writes every element of y."""
    import jax
    from jax.sharding import Mesh, PartitionSpec
    from jax.experimental.shard_map import shard_map

    from concourse import bass2jax
    from concourse.bass2jax import _bass_exec_p

    bass2jax.install_neuronx_cc_hook()
    part_name = nc.partition_id_tensor.name if nc.partition_id_tensor else None

    in_names, out_names, out_avals = [], [], []
    for alloc in nc.m.functions[0].allocations:
        if not isinstance(alloc, mybir.MemoryLocationSet):
            continue
        name = alloc.memorylocations[0].name
        if alloc.kind == "ExternalInput":
            if name != part_name:
                in_names.append(name)
        elif alloc.kind == "ExternalOutput":
            shape = tuple(alloc.tensor_shape)
            dtype = mybir.dt.np(alloc.dtype)
            out_names.append(name)
            out_avals.append(jax.core.ShapedArray(shape, dtype))
    n_params = len(in_names)
    all_names = in_names + out_names
    if part_name is not None:
        all_names = all_names + [part_name]

    def _body(*args):
        operands = list(args)
        if part_name is not None:
            operands.append(bass2jax.partition_id_tensor())
        outs = _bass_exec_p.bind(
            *operands,
            out_avals=tuple(out_avals),
            in_names=tuple(all_names),
            out_names=tuple(out_names),
            lowering_input_output_aliases=(),
            sim_require_finite=True,
            sim_require_nnan=True,
            nc=nc,
        )
        return tuple(outs)

    devices = jax.devices()[:n_cores]
    mesh = Mesh(np.asarray(devices), ("core",))
    n_out = len(out_names)
    sharded = jax.jit(
        shard_map(
            _body,
            mesh=mesh,
            in_specs=(PartitionSpec("core"),) * (n_params + n_out),
            out_specs=(PartitionSpec("core"),) * n_out,
            check_rep=False,
        ),
        keep_unused=True,
    )
    return sharded, in_names, out_names, out_avals


def run_timed(nc, in_maps, iters=20):
    """Execute on HW repeatedly with device-resident args; returns
    (per-core results, sorted per-call walls in seconds)."""
    import time

    import jax

    n_cores = len(in_maps)
    sharded, in_names, out_names, out_avals = _build_sharded_nodonate(nc, n_cores)
    concat_in = [
        np.concatenate([np.asarray(m[name]) for m in in_maps], axis=0)
        for name in in_names
    ]
    concat_zero = [
        np.zeros((n_cores * a.shape[0], *a.shape[1:]), a.dtype) for a in out_avals
    ]
    args = [jax.device_put(a) for a in concat_in + concat_zero]
    out = sharded(*args)  # compile + first run
    jax.block_until_ready(out)
    walls = []
    for _ in range(iters):
        t0 = time.perf_counter()
        out2 = sharded(*args)
        jax.block_until_ready(out2)
        walls.append(time.perf_counter() - t0)
    results = [
        {
            name: np.asarray(out[i]).reshape(n_cores, *out_avals[i].shape)[c]
            for i, name in enumerate(out_names)
        }
        for c in range(n_cores)
    ]
    return results, sorted(walls)


def baseline_rtt(iters=20):
    """Axon dispatch floor: same path with a trivial 8-core kernel."""
    nc = bacc.Bacc("TRN2", target_bir_lowering=False, debug=False, num_devices=8)
    a = nc.dram_tensor("a", [128, 128], F32, kind="ExternalInput")
    b = nc.dram_tensor("b", [128, 128], F32, kind="ExternalOutput")
    a, b = a.ap(), b.ap()
    with tile.TileContext(nc) as tc:
        with tc.tile_pool(name="p", bufs=1) as p:
            t = p.tile([128, 128], F32)
            nc.sync.dma_start(out=t, in_=a)
            nc.scalar.mul(out=t, in_=t, mul=2.0)
            nc.sync.dma_start(out=b, in_=t)
    nc.compile()
    in_maps = [{"a": np.zeros((128, 128), np.float32)} for _ in range(8)]
    _, walls = run_timed(nc, in_maps, iters=iters)
    return walls


def kernel(x, wq, bq, wk, bk, wv, bv, wo, bo, _trace=False, _trace_kwargs=None):
    nc = get_nc()
    in_maps = make_in_maps(x, wq, bq, wk, bk, wv, bv, wo, bo)
    res = run_bass_kernel_spmd(
        nc, in_maps, list(range(8)), trace=_trace, **(_trace_kwargs or {})
    )
    bo = np.asarray(bo, np.float32)
    out = np.empty((B, T, E), np.float32)
    for b in range(B):
        acc = res.results[b * 4]["y"].astype(np.float32).copy()
        for g in range(1, 4):
            acc += res.results[b * 4 + g]["y"]
        out[b] = acc + bo
    if _trace:
        return out, res
    return out


# revision 30
# speedup vs baseline: 64.7207x; 64.7207x over previous
"""Causal self-attention Trainium2 kernel (B=2, T=4096, E=768, H=12, D=64).

Sharding: 8 cores = 2 batches x 4 head-groups (3 heads each). Each core:
  - computes q/k in transposed layout [d, t] and v in natural layout [t, d]
    for its 3 heads (fp32r matmuls, PE transposes for x^T),
  - causal attention in S^T layout ([key, query] tiles) so softmax
    normalization needs no P transposes: exp on ACT, denominator via an
    extra ones-column appended to v (PV matmul row 64 = sum of exp),
  - normalizes via reciprocal + PE broadcast, out-projects with its wo
    row-slice producing a partial y [4096, 768].
Host sums the 4 partials per batch and adds bo.
"""

import os
import sys

sys.path.insert(0, "/opt/trn_rl_repo")

import numpy as np

try:  # persistent jit cache: skips the ~10min neuronxcc compile on re-runs
    import jax

    jax.config.update("jax_compilation_cache_dir", "/tmp/jax_neff_cache")
    jax.config.update("jax_persistent_cache_min_compile_time_secs", 10)
    jax.config.update("jax_persistent_cache_min_entry_size_bytes", 0)
except Exception:
    pass

import concourse.bass as bass
import concourse.mybir as mybir
import concourse.tile as tile
from concourse import bacc
from concourse.bass_utils import run_bass_kernel_spmd

F32 = mybir.dt.float32
F32R = mybir.dt.float32r

B, T, E, H = 2, 4096, 768, 12
D = E // H            # 64
HL = 3                # heads per core
CH = HL * D           # 192 channels per core
SB = 512              # query superblock
KB = 128              # key block
NEB = E // 128        # 6 embed tiles
SCALE = 1.0 / np.sqrt(D)

USE_F32R = os.environ.get("ATTN_NO_F32R", "") == ""


def _mm(ap):
    return ap.bitcast(F32R) if USE_F32R else ap


def build_nc(t_len=T, repeat=1):
    assert t_len % SB == 0
    nsb = t_len // SB       # superblocks
    ntb = t_len // KB       # 128-blocks

    nc = bacc.Bacc("TRN2", target_bir_lowering=False, debug=False, num_devices=8)

    xb = nc.dram_tensor("xb", [t_len, E], F32, kind="ExternalInput")
    wqk = nc.dram_tensor("wqk", [E, 2 * CH], F32, kind="ExternalInput")
    wvp = nc.dram_tensor("wvp", [E, 256], F32, kind="ExternalInput")
    wo = nc.dram_tensor("wo", [CH, E], F32, kind="ExternalInput")
    bqk = nc.dram_tensor("bqk", [HL, 2, D], F32, kind="ExternalInput")
    bv = nc.dram_tensor("bv", [CH + HL], F32, kind="ExternalInput")
    cst = nc.dram_tensor("cst", [128, 192], F32, kind="ExternalInput")
    y = nc.dram_tensor("y", [t_len, E], F32, kind="ExternalOutput")

    xb, wqk, wvp, wo, bqk, bv, cst, y = (
        t.ap() for t in (xb, wqk, wvp, wo, bqk, bv, cst, y)
    )

    with tile.TileContext(nc) as tc:
        import contextlib

        ctx = contextlib.ExitStack()
        with ctx:
            ctx.enter_context(
                nc.allow_low_precision(reason="fp32r rounding of matmul operands")
            )
            const = ctx.enter_context(tc.tile_pool(name="const", bufs=1))
            persist = ctx.enter_context(tc.tile_pool(name="persist", bufs=1))
            xpool = ctx.enter_context(tc.tile_pool(name="xpool", bufs=2))
            xtpool = ctx.enter_context(tc.tile_pool(name="xtpool", bufs=2))
            qspool = ctx.enter_context(tc.tile_pool(name="qspool", bufs=4))
            ospool = ctx.enter_context(tc.tile_pool(name="ospool", bufs=2))
            ptpool = ctx.enter_context(tc.tile_pool(name="ptpool", bufs=3))
            rpool = ctx.enter_context(tc.tile_pool(name="rpool", bufs=2))
            ypool = ctx.enter_context(tc.tile_pool(name="ypool", bufs=2))
            psA = ctx.enter_context(tc.tile_pool(name="psA", bufs=2, space="PSUM"))
            psS = ctx.enter_context(tc.tile_pool(name="psS", bufs=2, space="PSUM"))
            psO = ctx.enter_context(tc.tile_pool(name="psO", bufs=2, space="PSUM"))

            # ---- constants / weights in SBUF ----
            # identity and the broadcast-ones row come in via DMA: walrus
            # requires f32r-typed producers for matmul operands and its ISA
            # rejects f32r memsets
            ident = const.tile([128, 128], F32)
            nc.sync.dma_start(out=_mm(ident), in_=_mm(cst[:, 0:128]))
            ones65 = const.tile([65, D], F32)
            nc.sync.dma_start(
                out=_mm(ones65[64:65, :]), in_=_mm(cst[64:65, 128 : 128 + D])
            )

            wqk_sb = const.tile([128, NEB, 2 * CH], F32)
            nc.sync.dma_start(
                out=_mm(wqk_sb), in_=_mm(wqk).rearrange("(n p) m -> p n m", p=128)
            )
            wv_sb = const.tile([128, NEB, 256], F32)
            nc.sync.dma_start(
                out=_mm(wv_sb), in_=_mm(wvp).rearrange("(n p) m -> p n m", p=128)
            )
            wo01_sb = const.tile([128, E], F32)
            nc.sync.dma_start(out=_mm(wo01_sb), in_=_mm(wo[0 : 2 * D, :]))
            wo2_sb = const.tile([D, E], F32)
            nc.sync.dma_start(out=_mm(wo2_sb), in_=_mm(wo[2 * D : CH, :]))
            bqk_sb = const.tile([D, HL, 2], F32)
            nc.sync.dma_start(out=bqk_sb, in_=bqk.rearrange("h q p -> p h q"))
            # k-bias copy living at partitions 64..127 (k rows of the packed
            # qk psum) so the staging add is partition-aligned
            bk64_sb = const.tile([128, HL], F32)
            nc.sync.dma_start(
                out=bk64_sb[D : 2 * D, :], in_=bqk[:, 1, :].rearrange("h p -> p h")
            )
            bv_bc = const.tile([128, CH + HL], F32)
            nc.sync.dma_start(
                out=bv_bc,
                in_=bass.AP(
                    tensor=bv.tensor, offset=bv.offset, ap=[[0, 128]] + list(bv.ap)
                ),
            )

            # persistent activations
            kT = [persist.tile([D, t_len], F32, name=f"kT{h}") for h in range(HL)]
            v_sb = persist.tile([128, ntb, HL, D + 1], F32)

            import contextlib as _cl

            loop_cm = tc.For_i(0, repeat, 1) if repeat > 1 else _cl.nullcontext()

            def phase3(cp, oS):
                # out-projection for t-blocks of superblock cp
                for tb in range(4):
                    tg = cp * 4 + tb
                    y_sb = ypool.tile([128, E], F32, tag="y_sb", name="y_sb")
                    oS01p, oS2p = oS
                    for half in range(2):
                        ps_y = psO.tile([128, 384], F32, tag="psO", name="ps_y")
                        nc.tensor.matmul(
                            ps_y,
                            lhsT=_mm(oS01p[:, tb * KB : (tb + 1) * KB]),
                            rhs=_mm(wo01_sb[:, half * 384 : (half + 1) * 384]),
                            start=True,
                            stop=False,
                        )
                        nc.tensor.matmul(
                            ps_y,
                            lhsT=_mm(oS2p[:, tb * KB : (tb + 1) * KB]),
                            rhs=_mm(wo2_sb[:, half * 384 : (half + 1) * 384]),
                            start=False,
                            stop=True,
                        )
                        nc.vector.tensor_copy(
                            out=y_sb[:, half * 384 : (half + 1) * 384], in_=ps_y
                        )
                    nc.sync.dma_start(
                        out=y[tg * KB : (tg + 1) * KB, :], in_=y_sb
                    )

            with loop_cm:
              for c in range(nsb):
                # ======== phase 1: x^T, q^T/k^T, v for tokens [c*SB, (c+1)*SB)
                x_nat = xpool.tile([128, 4, E], F32, tag="x")
                for tb in range(4):
                    t0 = c * SB + tb * KB
                    nc.sync.dma_start(
                        out=_mm(x_nat[:, tb, :]), in_=_mm(xb[t0 : t0 + KB, :])
                    )
                xT = xtpool.tile([128, NEB, SB], F32, tag="xT")
                for eb in range(NEB):
                    ps_t = psA.tile([128, SB], F32, tag="psA", name="ps_t")
                    for tb in range(4):
                        nc.tensor.transpose(
                            _mm(ps_t[:, tb * 128 : (tb + 1) * 128]),
                            _mm(x_nat[:, tb, eb * 128 : (eb + 1) * 128]),
                            _mm(ident),
                        )
                    nc.vector.tensor_copy(out=_mm(xT[:, eb, :]), in_=ps_t)
                qS = []
                for h in range(HL):
                    ps_qk = psA.tile([128, SB], F32, tag="psA", name="ps_qk")
                    for eb in range(NEB):
                        nc.tensor.matmul(
                            ps_qk,
                            lhsT=_mm(wqk_sb[:, eb, h * 128 : (h + 1) * 128]),
                            rhs=_mm(xT[:, eb, :]),
                            start=(eb == 0),
                            stop=(eb == NEB - 1),
                        )
                    q_h = qspool.tile([D, SB], F32, tag="qS", name="q_h")
                    nc.vector.tensor_scalar_add(
                        out=_mm(q_h), in0=ps_qk[0:D, :], scalar1=bqk_sb[:, h, 0:1]
                    )
                    qS.append(q_h)
                    # k rows live at psum partitions 64..127. Lane engines
                    # cannot shift partitions, so stage at the same partitions
                    # (adding bias) and let an SBUF->SBUF DMA move them to
                    # partition base 0 in kT.
                    kst = qspool.tile([128, SB], F32, tag="kst", name="kst", bufs=2)
                    nc.vector.tensor_scalar_add(
                        out=_mm(kst[D : 2 * D, :]),
                        in0=ps_qk[D : 2 * D, :],
                        scalar1=bk64_sb[D : 2 * D, h : h + 1],
                    )
                    nc.sync.dma_start(
                        out=_mm(kT[h][:, c * SB : (c + 1) * SB]),
                        in_=_mm(kst[D : 2 * D, :]),
                    )
                for tb in range(4):
                    ps_v = psA.tile([128, 256], F32, tag="psA", name="ps_v")
                    for eb in range(NEB):
                        nc.tensor.matmul(
                            ps_v,
                            lhsT=_mm(xT[:, eb, tb * 128 : (tb + 1) * 128]),
                            rhs=_mm(wv_sb[:, eb, :]),
                            start=(eb == 0),
                            stop=(eb == NEB - 1),
                        )
                    nc.vector.tensor_add(
                        out=_mm(v_sb[:, c * 4 + tb, :, 0:D]),
                        in0=ps_v[:, 0:CH].rearrange("p (h d) -> p h d", h=HL),
                        in1=bv_bc[:, 0:CH].rearrange("p (h d) -> p h d", h=HL),
                    )
                    # ones column of v_aug: psum cols CH..CH+2 are x @ 0 = 0,
                    # plus the ones carried in the padded bias
                    nc.vector.tensor_add(
                        out=_mm(v_sb[:, c * 4 + tb, :, D : D + 1]),
                        in0=ps_v[:, CH : CH + HL].rearrange(
                            "p (h o) -> p h o", o=1
                        ),
                        in1=bv_bc[:, CH : CH + HL].rearrange(
                            "p (h o) -> p h o", o=1
                        ),
                    )

                # phase 3 of previous superblock goes here: its inputs (oS)
                # are produced by a DVE chain that lags PE, so slot the
                # already-runnable phase-1 work of this chunk in front of it
                if c > 0:
                    phase3(c - 1, oS_prev)

                # ======== phase 2: attention superblock i=c, all local heads
                nj = 4 * c + 4
                oS01 = ospool.tile([128, SB], F32, tag="oS01", name="oS01")
                oS2 = ospool.tile([D, SB], F32, tag="oS2", name="oS2")
                oS_prev_local = (oS01, oS2)
                for h in range(HL):
                    ps_o = psO.tile([128, SB], F32, tag="psO", name="ps_o")
                    q_ap = _mm(qS[h])

                    def q0_of(j):
                        # causal slice: key block j only sees queries
                        # >= j*KB - c*SB; keep the moving dim >= 256 so
                        # fp32r stays at full rate
                        if j < 4 * c:
                            return 0
                        return min((j - 4 * c) * KB, SB - 256)

                    def pv_step(j, pt_ap):
                        q0 = q0_of(j)
                        nc.tensor.matmul(
                            ps_o[0 : D + 1, q0:],
                            lhsT=_mm(v_sb[:, j, h, :]),
                            rhs=_mm(pt_ap[:, q0:]),
                            start=(j == 0),
                            stop=(j == nj - 1),
                        )

                    # process key blocks in pairs sharing one PSUM tile and
                    # (where widths match) a single wide exp on ACT, with a
                    # 2-block-deep software pipeline so PE stays ahead of ACT
                    pend = []
                    for jp in range(nj // 2):
                        j0, j1 = 2 * jp, 2 * jp + 1
                        q00, q01 = q0_of(j0), q0_of(j1)
                        ps_s2 = psS.tile([128, 2, SB], F32, tag="psS", name="ps_s2")
                        pt2 = ptpool.tile([128, 2, SB], F32, tag="pt", name="pt2")
                        for half, (j, q0) in enumerate(((j0, q00), (j1, q01))):
                            nc.tensor.matmul(
                                ps_s2[:, half, q0:],
                                lhsT=_mm(kT[h][:, j * KB : (j + 1) * KB]),
                                rhs=q_ap[:, q0:],
                                start=True,
                                stop=True,
                            )
                        if q00 == q01:
                            nc.scalar.activation(
                                out=_mm(pt2[:, :, q00:]),
                                in_=ps_s2[:, :, q00:],
                                func=mybir.ActivationFunctionType.Exp,
                                scale=float(SCALE),
                            )
                        else:
                            for half, q0 in ((0, q00), (1, q01)):
                                nc.scalar.activation(
                                    out=_mm(pt2[:, half, q0:]),
                                    in_=ps_s2[:, half, q0:],
                                    func=mybir.ActivationFunctionType.Exp,
                                    scale=float(SCALE),
                                )
                        for half, (j, q0) in enumerate(((j0, q00), (j1, q01))):
                            if j >= 4 * c:
                                nc.gpsimd.affine_select(
                                    out=_mm(pt2[:, half, q0:]),
                                    in_=_mm(pt2[:, half, q0:]),
                                    compare_op=mybir.AluOpType.is_ge,
                                    fill=0.0,
                                    base=c * SB - j * KB + q0,
                                    pattern=[[1, SB - q0]],
                                    channel_multiplier=-1,
                                )
                        pend.append((j0, pt2[:, 0, :]))
                        pend.append((j1, pt2[:, 1, :]))
                        while len(pend) > 4:
                            pv_step(*pend.pop(0))
                    for jq in pend:
                        pv_step(*jq)

                    recip = rpool.tile([65, SB], F32, tag="recip", name="recip")
                    nc.vector.reciprocal(_mm(recip[64:65, :]), ps_o[D : D + 1, :])
                    ps_b = psS.tile([128, SB], F32, tag="psS", name="ps_b")
                    nc.tensor.matmul(
                        ps_b[0:D, :],
                        lhsT=_mm(ones65[64:65, :]),
                        rhs=_mm(recip[64:65, :]),
                        start=True,
                        stop=True,
                    )
                    # walrus: a DVE op may read only ONE non-scalar PSUM input,
                    # so stage the broadcast row in SBUF before the multiply
                    rb = rpool.tile([D, SB], F32, tag="rbcast", name="rb")
                    nc.vector.tensor_copy(out=rb, in_=ps_b[0:D, :])
                    if h == 0:
                        o_dst = oS01[0:D, :]
                    elif h == 2:
                        o_dst = oS2[:, :]
                    else:
                        o_dst = ospool.tile(
                            [D, SB], F32, tag="o1tmp", name="o1tmp"
                        )
                    nc.vector.tensor_mul(_mm(o_dst), ps_o[0:D, :], rb)
                    if h == 1:
                        # stack h1 under h0 (partitions 64:128) via DMA, the
                        # only engine that can shift partitions
                        nc.sync.dma_start(out=_mm(oS01[D : 2 * D, :]), in_=_mm(o_dst))
                oS_prev = oS_prev_local
              phase3(nsb - 1, oS_prev)
    nc.compile()
    return nc


def make_in_maps(x, wq, bq, wk, bk, wv, bv, wo, bo, t_len=T):
    x = np.asarray(x, np.float32)
    in_maps = []
    for c in range(8):
        b, g = divmod(c, 4)
        hs = slice(g * CH, (g + 1) * CH)
        wqk_c = np.empty((E, 2 * CH), np.float32)
        bqk_c = np.empty((HL, 2, D), np.float32)
        for hl in range(HL):
            h = g * HL + hl
            wqk_c[:, hl * 128 : hl * 128 + D] = wq[:, h * D : (h + 1) * D]
            wqk_c[:, hl * 128 + D : (hl + 1) * 128] = wk[:, h * D : (h + 1) * D]
            bqk_c[hl, 0] = bq[h * D : (h + 1) * D]
            bqk_c[hl, 1] = bk[h * D : (h + 1) * D]
        wv_c = np.zeros((E, 256), np.float32)
        wv_c[:, :CH] = wv[:, hs]
        bv_c = np.ones(CH + HL, np.float32)
        bv_c[:CH] = np.asarray(bv, np.float32)[hs]
        cst = np.concatenate(
            [np.eye(128, dtype=np.float32), np.ones((128, 64), np.float32)], axis=1
        )
        in_maps.append(
            {
                "xb": np.ascontiguousarray(x[b, :t_len]),
                "wqk": wqk_c,
                "wvp": wv_c,
                "wo": np.ascontiguousarray(np.asarray(wo, np.float32)[hs]),
                "bqk": bqk_c,
                "bv": bv_c,
                "cst": cst,
            }
        )
    return in_maps


_NC_CACHE = {}


def get_nc(t_len=T):
    if t_len not in _NC_CACHE:
        _NC_CACHE[t_len] = build_nc(t_len)
    return _NC_CACHE[t_len]


def _build_sharded_nodonate(nc, n_cores=8):
    """Mirror bass2jax.run_bass_via_pjrt's multi-core path, minus donation,
    returning (jitted_fn, in_names, out_names, out_avals). Without donation a
    call can be repeated on device-resident arrays for timing. Safe here: the
    kernel writes every element of y."""
    import jax
    from jax.sharding import Mesh, PartitionSpec
    from jax.experimental.shard_map import shard_map

    from concourse import bass2jax
    from concourse.bass2jax import _bass_exec_p

    bass2jax.install_neuronx_cc_hook()
    part_name = nc.partition_id_tensor.name if nc.partition_id_tensor else None

    in_names, out_names, out_avals = [], [], []
    for alloc in nc.m.functions[0].allocations:
        if not isinstance(alloc, mybir.MemoryLocationSet):
            continue
        name = alloc.memorylocations[0].name
        if alloc.kind == "ExternalInput":
            if name != part_name:
                in_names.append(name)
        elif alloc.kind == "ExternalOutput":
            shape = tuple(alloc.tensor_shape)
            dtype = mybir.dt.np(alloc.dtype)
            out_names.append(name)
            out_avals.append(jax.core.ShapedArray(shape, dtype))
    n_params = len(in_names)
    all_names = in_names + out_names
    if part_name is not None:
        all_names = all_names + [part_name]

    def _body(*args):
        operands = list(args)
        if part_name is not None:
            operands.append(bass2jax.partition_id_tensor())
        outs = _bass_exec_p.bind(
            *operands,
            out_avals=tuple(out_avals),
            in_names=tuple(all_names),
            out_names=tuple(out_names),
            lowering_input_output_aliases=(),
            sim_require_finite=True,
            sim_require_nnan=True,
            nc=nc,
        )
        return tuple(outs)

    devices = jax.devices()[:n_cores]
    mesh = Mesh(np.asarray(devices), ("core",))
    n_out = len(out_names)
    sharded = jax.jit(
        shard_map(
            _body,
            mesh=mesh,
            in_specs=(PartitionSpec("core"),) * (n_params + n_out),
            out_specs=(PartitionSpec("core"),) * n_out,
            check_rep=False,
        ),
        keep_unused=True,
    )
    return sharded, in_names, out_names, out_avals


def run_timed(nc, in_maps, iters=20):
    """Execute on HW repeatedly with device-resident args; returns
    (per-core results, sorted per-call walls in seconds)."""
    import time

    import jax

    n_cores = len(in_maps)
    sharded, in_names, out_names, out_avals = _build_sharded_nodonate(nc, n_cores)
    concat_in = [
        np.concatenate([np.asarray(m[name]) for m in in_maps], axis=0)
        for name in in_names
    ]
    concat_zero = [
        np.zeros((n_cores * a.shape[0], *a.shape[1:]), a.dtype) for a in out_avals
    ]
    args = [jax.device_put(a) for a in concat_in + concat_zero]
    out = sharded(*args)  # compile + first run
    jax.block_until_ready(out)
    walls = []
    for _ in range(iters):
        t0 = time.perf_counter()
        out2 = sharded(*args)
        jax.block_until_ready(out2)
        walls.append(time.perf_counter() - t0)
    results = [
        {
            name: np.asarray(out[i]).reshape(n_cores, *out_avals[i].shape)[c]
            for i, name in enumerate(out_names)
        }
        for c in range(n_cores)
    ]
    return results, sorted(walls)


def baseline_rtt(iters=20):
    """Axon dispatch floor: same path with a trivial 8-core kernel."""
    nc = bacc.Bacc("TRN2", target_bir_lowering=False, debug=False, num_devices=8)
    a = nc.dram_tensor("a", [128, 128], F32, kind="ExternalInput")
    b = nc.dram_tensor("b", [128, 128], F32, kind="ExternalOutput")
    a, b = a.ap(), b.ap()
    with tile.TileContext(nc) as tc:
        with tc.tile_pool(name="p", bufs=1) as p:
            t = p.tile([128, 128], F32)
            nc.sync.dma_start(out=t, in_=a)
            nc.scalar.mul(out=t, in_=t, mul=2.0)
            nc.sync.dma_start(out=b, in_=t)
    nc.compile()
    in_maps = [{"a": np.zeros((128, 128), np.float32)} for _ in range(8)]
    _, walls = run_timed(nc, in_maps, iters=iters)
    return walls


def kernel(x, wq, bq, wk, bk, wv, bv, wo, bo, _trace=False, _trace_kwargs=None):
    nc = get_nc()
    in_maps = make_in_maps(x, wq, bq, wk, bk, wv, bv, wo, bo)
    res = run_bass_kernel_spmd(
        nc, in_maps, list(range(8)), trace=_trace, **(_trace_kwargs or {})
    )
    bo = np.asarray(bo, np.float32)
    out = np.empty((B, T, E), np.float32)
    for b in range(B):
        acc = res.results[b * 4]["y"].astype(np.float32).copy()
        for g in range(1, 4):
            acc += res.results[b * 4 + g]["y"]
        out[b] = acc + bo
    if _trace:
        return out, res
    return out


# revision 31
# speedup vs baseline: 69.7528x; 1.0778x over previous
"""Causal self-attention Trainium2 kernel (B=2, T=4096, E=768, H=12, D=64).

Sharding: 8 cores = 2 batches x 4 head-groups (3 heads each). Each core:
  - computes q/k in transposed layout [d, t] and v in natural layout [t, d]
    for its 3 heads (fp32r matmuls, PE transposes for x^T),
  - causal attention in S^T layout ([key, query] tiles) so softmax
    normalization needs no P transposes: exp on ACT, denominator via an
    extra ones-column appended to v (PV matmul row 64 = sum of exp),
  - normalizes via reciprocal + PE broadcast, out-projects with its wo
    row-slice producing a partial y [4096, 768].
Host sums the 4 partials per batch and adds bo.
"""

import os
import sys

sys.path.insert(0, "/opt/trn_rl_repo")

import numpy as np

try:  # persistent jit cache: skips the ~10min neuronxcc compile on re-runs
    import jax

    jax.config.update("jax_compilation_cache_dir", "/tmp/jax_neff_cache")
    jax.config.update("jax_persistent_cache_min_compile_time_secs", 10)
    jax.config.update("jax_persistent_cache_min_entry_size_bytes", 0)
except Exception:
    pass

import concourse.bass as bass
import concourse.mybir as mybir
import concourse.tile as tile
from concourse import bacc
from concourse.bass_utils import run_bass_kernel_spmd

F32 = mybir.dt.float32
F32R = mybir.dt.float32r

B, T, E, H = 2, 4096, 768, 12
D = E // H            # 64
HL = 3                # heads per core
CH = HL * D           # 192 channels per core
SB = 512              # query superblock
KB = 128              # key block
NEB = E // 128        # 6 embed tiles
SCALE = 1.0 / np.sqrt(D)

USE_F32R = os.environ.get("ATTN_NO_F32R", "") == ""
USE_BF16 = os.environ.get("ATTN_BF16", "") != ""
BF16 = mybir.dt.bfloat16
# attention-operand dtype (k/q/exp(S)/v tiles)
AD = BF16 if USE_BF16 else F32


def _mm(ap):
    return ap.bitcast(F32R) if USE_F32R else ap


def _att(ap):
    # bf16 tiles are natively matmul-legal; fp32 tiles need the f32r bitcast
    return ap if USE_BF16 else _mm(ap)


def build_nc(t_len=T, repeat=1):
    assert t_len % SB == 0
    nsb = t_len // SB       # superblocks
    ntb = t_len // KB       # 128-blocks

    nc = bacc.Bacc("TRN2", target_bir_lowering=False, debug=False, num_devices=8)

    xb = nc.dram_tensor("xb", [t_len, E], F32, kind="ExternalInput")
    wqk = nc.dram_tensor("wqk", [E, 2 * CH], F32, kind="ExternalInput")
    wvp = nc.dram_tensor("wvp", [E, 256], F32, kind="ExternalInput")
    wo = nc.dram_tensor("wo", [CH, E], F32, kind="ExternalInput")
    bqk = nc.dram_tensor("bqk", [HL, 2, D], F32, kind="ExternalInput")
    bv = nc.dram_tensor("bv", [CH + HL], F32, kind="ExternalInput")
    cst = nc.dram_tensor("cst", [128, 192], F32, kind="ExternalInput")
    y = nc.dram_tensor("y", [t_len, E], F32, kind="ExternalOutput")

    xb, wqk, wvp, wo, bqk, bv, cst, y = (
        t.ap() for t in (xb, wqk, wvp, wo, bqk, bv, cst, y)
    )

    with tile.TileContext(nc) as tc:
        import contextlib

        ctx = contextlib.ExitStack()
        with ctx:
            ctx.enter_context(
                nc.allow_low_precision(reason="fp32r rounding of matmul operands")
            )
            const = ctx.enter_context(tc.tile_pool(name="const", bufs=1))
            persist = ctx.enter_context(tc.tile_pool(name="persist", bufs=1))
            xpool = ctx.enter_context(tc.tile_pool(name="xpool", bufs=2))
            xtpool = ctx.enter_context(tc.tile_pool(name="xtpool", bufs=2))
            qspool = ctx.enter_context(tc.tile_pool(name="qspool", bufs=4))
            ospool = ctx.enter_context(tc.tile_pool(name="ospool", bufs=2))
            ptpool = ctx.enter_context(tc.tile_pool(name="ptpool", bufs=3))
            rpool = ctx.enter_context(tc.tile_pool(name="rpool", bufs=2))
            ypool = ctx.enter_context(tc.tile_pool(name="ypool", bufs=2))
            psA = ctx.enter_context(tc.tile_pool(name="psA", bufs=2, space="PSUM"))
            psS = ctx.enter_context(tc.tile_pool(name="psS", bufs=2, space="PSUM"))
            psO = ctx.enter_context(tc.tile_pool(name="psO", bufs=2, space="PSUM"))

            # ---- constants / weights in SBUF ----
            # identity and the broadcast-ones row come in via DMA: walrus
            # requires f32r-typed producers for matmul operands and its ISA
            # rejects f32r memsets
            ident = const.tile([128, 128], F32)
            nc.sync.dma_start(out=_mm(ident), in_=_mm(cst[:, 0:128]))
            ones65 = const.tile([65, D], F32)
            nc.sync.dma_start(
                out=_mm(ones65[64:65, :]), in_=_mm(cst[64:65, 128 : 128 + D])
            )

            wqk_sb = const.tile([128, NEB, 2 * CH], F32)
            nc.sync.dma_start(
                out=_mm(wqk_sb), in_=_mm(wqk).rearrange("(n p) m -> p n m", p=128)
            )
            wv_sb = const.tile([128, NEB, 256], F32)
            nc.sync.dma_start(
                out=_mm(wv_sb), in_=_mm(wvp).rearrange("(n p) m -> p n m", p=128)
            )
            wo01_sb = const.tile([128, E], F32)
            nc.sync.dma_start(out=_mm(wo01_sb), in_=_mm(wo[0 : 2 * D, :]))
            wo2_sb = const.tile([D, E], F32)
            nc.sync.dma_start(out=_mm(wo2_sb), in_=_mm(wo[2 * D : CH, :]))
            bqk_sb = const.tile([D, HL, 2], F32)
            nc.sync.dma_start(out=bqk_sb, in_=bqk.rearrange("h q p -> p h q"))
            # k-bias copy living at partitions 64..127 (k rows of the packed
            # qk psum) so the staging add is partition-aligned
            bk64_sb = const.tile([128, HL], F32)
            nc.sync.dma_start(
                out=bk64_sb[D : 2 * D, :], in_=bqk[:, 1, :].rearrange("h p -> p h")
            )
            bv_bc = const.tile([128, CH + HL], F32)
            nc.sync.dma_start(
                out=bv_bc,
                in_=bass.AP(
                    tensor=bv.tensor, offset=bv.offset, ap=[[0, 128]] + list(bv.ap)
                ),
            )

            # persistent activations
            kT = [persist.tile([D, t_len], AD, name=f"kT{h}") for h in range(HL)]
            v_sb = persist.tile([128, ntb, HL, D + 1], AD)

            import contextlib as _cl

            loop_cm = tc.For_i(0, repeat, 1) if repeat > 1 else _cl.nullcontext()

            def phase3(cp, oS):
                # out-projection for t-blocks of superblock cp
                for tb in range(4):
                    tg = cp * 4 + tb
                    y_sb = ypool.tile([128, E], F32, tag="y_sb", name="y_sb")
                    oS01p, oS2p = oS
                    for half in range(2):
                        ps_y = psO.tile([128, 384], F32, tag="psO", name="ps_y")
                        nc.tensor.matmul(
                            ps_y,
                            lhsT=_mm(oS01p[:, tb * KB : (tb + 1) * KB]),
                            rhs=_mm(wo01_sb[:, half * 384 : (half + 1) * 384]),
                            start=True,
                            stop=False,
                        )
                        nc.tensor.matmul(
                            ps_y,
                            lhsT=_mm(oS2p[:, tb * KB : (tb + 1) * KB]),
                            rhs=_mm(wo2_sb[:, half * 384 : (half + 1) * 384]),
                            start=False,
                            stop=True,
                        )
                        nc.vector.tensor_copy(
                            out=y_sb[:, half * 384 : (half + 1) * 384], in_=ps_y
                        )
                    nc.sync.dma_start(
                        out=y[tg * KB : (tg + 1) * KB, :], in_=y_sb
                    )

            with loop_cm:
              for c in range(nsb):
                # ======== phase 1: x^T, q^T/k^T, v for tokens [c*SB, (c+1)*SB)
                x_nat = xpool.tile([128, 4, E], F32, tag="x")
                for tb in range(4):
                    t0 = c * SB + tb * KB
                    nc.sync.dma_start(
                        out=_mm(x_nat[:, tb, :]), in_=_mm(xb[t0 : t0 + KB, :])
                    )
                xT = xtpool.tile([128, NEB, SB], F32, tag="xT")
                for eb in range(NEB):
                    ps_t = psA.tile([128, SB], F32, tag="psA", name="ps_t")
                    for tb in range(4):
                        nc.tensor.transpose(
                            _mm(ps_t[:, tb * 128 : (tb + 1) * 128]),
                            _mm(x_nat[:, tb, eb * 128 : (eb + 1) * 128]),
                            _mm(ident),
                        )
                    nc.vector.tensor_copy(out=_mm(xT[:, eb, :]), in_=ps_t)
                qS = []
                for h in range(HL):
                    ps_qk = psA.tile([128, SB], F32, tag="psA", name="ps_qk")
                    for eb in range(NEB):
                        nc.tensor.matmul(
                            ps_qk,
                            lhsT=_mm(wqk_sb[:, eb, h * 128 : (h + 1) * 128]),
                            rhs=_mm(xT[:, eb, :]),
                            start=(eb == 0),
                            stop=(eb == NEB - 1),
                        )
                    q_h = qspool.tile([D, SB], AD, tag="qS", name="q_h")
                    nc.vector.tensor_scalar_add(
                        out=_att(q_h), in0=ps_qk[0:D, :], scalar1=bqk_sb[:, h, 0:1]
                    )
                    qS.append(q_h)
                    # k rows live at psum partitions 64..127. Lane engines
                    # cannot shift partitions, so stage at the same partitions
                    # (adding bias) and let an SBUF->SBUF DMA move them to
                    # partition base 0 in kT.
                    kst = qspool.tile([128, SB], AD, tag="kst", name="kst", bufs=2)
                    nc.vector.tensor_scalar_add(
                        out=_att(kst[D : 2 * D, :]),
                        in0=ps_qk[D : 2 * D, :],
                        scalar1=bk64_sb[D : 2 * D, h : h + 1],
                    )
                    nc.sync.dma_start(
                        out=_att(kT[h][:, c * SB : (c + 1) * SB]),
                        in_=_att(kst[D : 2 * D, :]),
                    )
                for tb in range(4):
                    ps_v = psA.tile([128, 256], F32, tag="psA", name="ps_v")
                    for eb in range(NEB):
                        nc.tensor.matmul(
                            ps_v,
                            lhsT=_mm(xT[:, eb, tb * 128 : (tb + 1) * 128]),
                            rhs=_mm(wv_sb[:, eb, :]),
                            start=(eb == 0),
                            stop=(eb == NEB - 1),
                        )
                    nc.vector.tensor_add(
                        out=_att(v_sb[:, c * 4 + tb, :, 0:D]),
                        in0=ps_v[:, 0:CH].rearrange("p (h d) -> p h d", h=HL),
                        in1=bv_bc[:, 0:CH].rearrange("p (h d) -> p h d", h=HL),
                    )
                    # ones column of v_aug: psum cols CH..CH+2 are x @ 0 = 0,
                    # plus the ones carried in the padded bias
                    nc.vector.tensor_add(
                        out=_att(v_sb[:, c * 4 + tb, :, D : D + 1]),
                        in0=ps_v[:, CH : CH + HL].rearrange(
                            "p (h o) -> p h o", o=1
                        ),
                        in1=bv_bc[:, CH : CH + HL].rearrange(
                            "p (h o) -> p h o", o=1
                        ),
                    )

                # phase 3 of previous superblock goes here: its inputs (oS)
                # are produced by a DVE chain that lags PE, so slot the
                # already-runnable phase-1 work of this chunk in front of it
                if c > 0:
                    phase3(c - 1, oS_prev)

                # ======== phase 2: attention superblock i=c, all local heads
                nj = 4 * c + 4
                oS01 = ospool.tile([128, SB], F32, tag="oS01", name="oS01")
                oS2 = ospool.tile([D, SB], F32, tag="oS2", name="oS2")
                oS_prev_local = (oS01, oS2)
                for h in range(HL):
                    ps_o = psO.tile([128, SB], F32, tag="psO", name="ps_o")
                    q_ap = _att(qS[h])

                    def q0_of(j):
                        # causal slice: key block j only sees queries
                        # >= j*KB - c*SB; keep the moving dim >= 256 so
                        # fp32r stays at full rate
                        if j < 4 * c:
                            return 0
                        return min((j - 4 * c) * KB, SB - 256)

                    def pv_step(j, pt_ap):
                        q0 = q0_of(j)
                        nc.tensor.matmul(
                            ps_o[0 : D + 1, q0:],
                            lhsT=_att(v_sb[:, j, h, :]),
                            rhs=_att(pt_ap[:, q0:]),
                            start=(j == 0),
                            stop=(j == nj - 1),
                        )

                    # process key blocks in pairs sharing one PSUM tile and
                    # (where widths match) a single wide exp on ACT, with a
                    # 2-block-deep software pipeline so PE stays ahead of ACT
                    pend = []
                    for jp in range(nj // 2):
                        j0, j1 = 2 * jp, 2 * jp + 1
                        q00, q01 = q0_of(j0), q0_of(j1)
                        ps_s2 = psS.tile([128, 2, SB], F32, tag="psS", name="ps_s2")
                        pt2 = ptpool.tile([128, 2, SB], AD, tag="pt", name="pt2")
                        for half, (j, q0) in enumerate(((j0, q00), (j1, q01))):
                            nc.tensor.matmul(
                                ps_s2[:, half, q0:],
                                lhsT=_att(kT[h][:, j * KB : (j + 1) * KB]),
                                rhs=q_ap[:, q0:],
                                start=True,
                                stop=True,
                            )
                        if q00 == q01:
                            nc.scalar.activation(
                                out=_att(pt2[:, :, q00:]),
                                in_=ps_s2[:, :, q00:],
                                func=mybir.ActivationFunctionType.Exp,
                                scale=float(SCALE),
                            )
                        else:
                            for half, q0 in ((0, q00), (1, q01)):
                                nc.scalar.activation(
                                    out=_att(pt2[:, half, q0:]),
                                    in_=ps_s2[:, half, q0:],
                                    func=mybir.ActivationFunctionType.Exp,
                                    scale=float(SCALE),
                                )
                        for half, (j, q0) in enumerate(((j0, q00), (j1, q01))):
                            if j >= 4 * c:
                                nc.gpsimd.affine_select(
                                    out=_att(pt2[:, half, q0:]),
                                    in_=_att(pt2[:, half, q0:]),
                                    compare_op=mybir.AluOpType.is_ge,
                                    fill=0.0,
                                    base=c * SB - j * KB + q0,
                                    pattern=[[1, SB - q0]],
                                    channel_multiplier=-1,
                                )
                        pend.append((j0, pt2[:, 0, :]))
                        pend.append((j1, pt2[:, 1, :]))
                        while len(pend) > 4:
                            pv_step(*pend.pop(0))
                    for jq in pend:
                        pv_step(*jq)

                    recip = rpool.tile([65, SB], F32, tag="recip", name="recip")
                    nc.vector.reciprocal(_mm(recip[64:65, :]), ps_o[D : D + 1, :])
                    ps_b = psS.tile([128, SB], F32, tag="psS", name="ps_b")
                    nc.tensor.matmul(
                        ps_b[0:D, :],
                        lhsT=_mm(ones65[64:65, :]),
                        rhs=_mm(recip[64:65, :]),
                        start=True,
                        stop=True,
                    )
                    # walrus: a DVE op may read only ONE non-scalar PSUM input,
                    # so stage the broadcast row in SBUF before the multiply
                    rb = rpool.tile([D, SB], F32, tag="rbcast", name="rb")
                    nc.vector.tensor_copy(out=rb, in_=ps_b[0:D, :])
                    if h == 0:
                        o_dst = oS01[0:D, :]
                    elif h == 2:
                        o_dst = oS2[:, :]
                    else:
                        o_dst = ospool.tile(
                            [D, SB], F32, tag="o1tmp", name="o1tmp"
                        )
                    nc.vector.tensor_mul(_mm(o_dst), ps_o[0:D, :], rb)
                    if h == 1:
                        # stack h1 under h0 (partitions 64:128) via DMA, the
                        # only engine that can shift partitions
                        nc.sync.dma_start(out=_mm(oS01[D : 2 * D, :]), in_=_mm(o_dst))
                oS_prev = oS_prev_local
              phase3(nsb - 1, oS_prev)
    nc.compile()
    return nc


def make_in_maps(x, wq, bq, wk, bk, wv, bv, wo, bo, t_len=T):
    x = np.asarray(x, np.float32)
    in_maps = []
    for c in range(8):
        b, g = divmod(c, 4)
        hs = slice(g * CH, (g + 1) * CH)
        wqk_c = np.empty((E, 2 * CH), np.float32)
        bqk_c = np.empty((HL, 2, D), np.float32)
        for hl in range(HL):
            h = g * HL + hl
            wqk_c[:, hl * 128 : hl * 128 + D] = wq[:, h * D : (h + 1) * D]
            wqk_c[:, hl * 128 + D : (hl + 1) * 128] = wk[:, h * D : (h + 1) * D]
            bqk_c[hl, 0] = bq[h * D : (h + 1) * D]
            bqk_c[hl, 1] = bk[h * D : (h + 1) * D]
        wv_c = np.zeros((E, 256), np.float32)
        wv_c[:, :CH] = wv[:, hs]
        bv_c = np.ones(CH + HL, np.float32)
        bv_c[:CH] = np.asarray(bv, np.float32)[hs]
        cst = np.concatenate(
            [np.eye(128, dtype=np.float32), np.ones((128, 64), np.float32)], axis=1
        )
        in_maps.append(
            {
                "xb": np.ascontiguousarray(x[b, :t_len]),
                "wqk": wqk_c,
                "wvp": wv_c,
                "wo": np.ascontiguousarray(np.asarray(wo, np.float32)[hs]),
                "bqk": bqk_c,
                "bv": bv_c,
                "cst": cst,
            }
        )
    return in_maps


_NC_CACHE = {}


def get_nc(t_len=T):
    if t_len not in _NC_CACHE:
        _NC_CACHE[t_len] = build_nc(t_len)
    return _NC_CACHE[t_len]


def _build_sharded_nodonate(nc, n_cores=8):
    """Mirror bass2jax.run_bass_via_pjrt's multi-core path, minus donation,
    returning (jitted_fn, in_names, out_names, out_avals). Without donation a
    call can be repeated on device-resident arrays for timing. Safe here: the
    kernel writes every element of y."""
    import jax
    from jax.sharding import Mesh, PartitionSpec
    from jax.experimental.shard_map import shard_map

    from concourse import bass2jax
    from concourse.bass2jax import _bass_exec_p

    bass2jax.install_neuronx_cc_hook()
    part_name = nc.partition_id_tensor.name if nc.partition_id_tensor else None

    in_names, out_names, out_avals = [], [], []
    for alloc in nc.m.functions[0].allocations:
        if not isinstance(alloc, mybir.MemoryLocationSet):
            continue
        name = alloc.memorylocations[0].name
        if alloc.kind == "ExternalInput":
            if name != part_name:
                in_names.append(name)
        elif alloc.kind == "ExternalOutput":
            shape = tuple(alloc.tensor_shape)
            dtype = mybir.dt.np(alloc.dtype)
            out_names.append(name)
            out_avals.append(jax.core.ShapedArray(shape, dtype))
    n_params = len(in_names)
    all_names = in_names + out_names
    if part_name is not None:
        all_names = all_names + [part_name]

    def _body(*args):
        operands = list(args)
        if part_name is not None:
            operands.append(bass2jax.partition_id_tensor())
        outs = _bass_exec_p.bind(
            *operands,
            out_avals=tuple(out_avals),
            in_names=tuple(all_names),
            out_names=tuple(out_names),
            lowering_input_output_aliases=(),
            sim_require_finite=True,
            sim_require_nnan=True,
            nc=nc,
        )
        return tuple(outs)

    devices = jax.devices()[:n_cores]
    mesh = Mesh(np.asarray(devices), ("core",))
    n_out = len(out_names)
    sharded = jax.jit(
        shard_map(
            _body,
            mesh=mesh,
            in_specs=(PartitionSpec("core"),) * (n_params + n_out),
            out_specs=(PartitionSpec("core"),) * n_out,
            check_rep=False,
        ),
        keep_unused=True,
    )
    return sharded, in_names, out_names, out_avals


def run_timed(nc, in_maps, iters=20):
    """Execute on HW repeatedly with device-resident args; returns
    (per-core results, sorted per-call walls in seconds)."""
    import time

    import jax

    n_cores = len(in_maps)
    sharded, in_names, out_names, out_avals = _build_sharded_nodonate(nc, n_cores)
    concat_in = [
        np.concatenate([np.asarray(m[name]) for m in in_maps], axis=0)
        for name in in_names
    ]
    concat_zero = [
        np.zeros((n_cores * a.shape[0], *a.shape[1:]), a.dtype) for a in out_avals
    ]
    args = [jax.device_put(a) for a in concat_in + concat_zero]
    out = sharded(*args)  # compile + first run
    jax.block_until_ready(out)
    walls = []
    for _ in range(iters):
        t0 = time.perf_counter()
        out2 = sharded(*args)
        jax.block_until_ready(out2)
        walls.append(time.perf_counter() - t0)
    results = [
        {
            name: np.asarray(out[i]).reshape(n_cores, *out_avals[i].shape)[c]
            for i, name in enumerate(out_names)
        }
        for c in range(n_cores)
    ]
    return results, sorted(walls)


def baseline_rtt(iters=20):
    """Axon dispatch floor: same path with a trivial 8-core kernel."""
    nc = bacc.Bacc("TRN2", target_bir_lowering=False, debug=False, num_devices=8)
    a = nc.dram_tensor("a", [128, 128], F32, kind="ExternalInput")
    b = nc.dram_tensor("b", [128, 128], F32, kind="ExternalOutput")
    a, b = a.ap(), b.ap()
    with tile.TileContext(nc) as tc:
        with tc.tile_pool(name="p", bufs=1) as p:
            t = p.tile([128, 128], F32)
            nc.sync.dma_start(out=t, in_=a)
            nc.scalar.mul(out=t, in_=t, mul=2.0)
            nc.sync.dma_start(out=b, in_=t)
    nc.compile()
    in_maps = [{"a": np.zeros((128, 128), np.float32)} for _ in range(8)]
    _, walls = run_timed(nc, in_maps, iters=iters)
    return walls


def kernel(x, wq, bq, wk, bk, wv, bv, wo, bo, _trace=False, _trace_kwargs=None):
    nc = get_nc()
    in_maps = make_in_maps(x, wq, bq, wk, bk, wv, bv, wo, bo)
    res = run_bass_kernel_spmd(
        nc, in_maps, list(range(8)), trace=_trace, **(_trace_kwargs or {})
    )
    bo = np.asarray(bo, np.float32)
    out = np.empty((B, T, E), np.float32)
    for b in range(B):
        acc = res.results[b * 4]["y"].astype(np.float32).copy()
        for g in range(1, 4):
            acc += res.results[b * 4 + g]["y"]
        out[b] = acc + bo
    if _trace:
        return out, res
    return out


# revision 35
# speedup vs baseline: 146.4417x; 2.0994x over previous
"""Causal self-attention Trainium2 kernel (B=2, T=4096, E=768, H=12, D=64).

Sharding: 8 cores = 2 batches x 4 head-groups (3 heads each). Each core:
  - computes q/k in transposed layout [d, t] and v in natural layout [t, d]
    for its 3 heads (fp32r matmuls, PE transposes for x^T),
  - causal attention in S^T layout ([key, query] tiles) so softmax
    normalization needs no P transposes: exp on ACT, denominator via an
    extra ones-column appended to v (PV matmul row 64 = sum of exp),
  - normalizes via reciprocal + PE broadcast, out-projects with its wo
    row-slice producing a partial y [4096, 768].
Host sums the 4 partials per batch and adds bo.

Matmuls run as float32r (4-byte storage, ~tf32 matmul precision, 4x the fp32
rate): measured 2.55e-4 relative(absmax) error vs the fp32 jax reference on
HW, ~690 us per invocation (8-core parallel) via the repeat-loop slope.
ATTN_* env flags are dev/diagnostic variants; defaults are the shipped path.
"""

import os
import sys

sys.path.insert(0, "/opt/trn_rl_repo")

import numpy as np

try:  # persistent jit cache: skips the ~10min neuronxcc compile on re-runs
    import jax

    jax.config.update("jax_compilation_cache_dir", "/tmp/jax_neff_cache")
    jax.config.update("jax_persistent_cache_min_compile_time_secs", 10)
    jax.config.update("jax_persistent_cache_min_entry_size_bytes", 0)
except Exception:
    pass

import concourse.bass as bass
import concourse.mybir as mybir
import concourse.tile as tile
from concourse import bacc
from concourse.bass_utils import run_bass_kernel_spmd

F32 = mybir.dt.float32
F32R = mybir.dt.float32r

B, T, E, H = 2, 4096, 768, 12
D = E // H            # 64
HL = 3                # heads per core
CH = HL * D           # 192 channels per core
SB = 512              # query superblock
KB = 128              # key block
NEB = E // 128        # 6 embed tiles
SCALE = 1.0 / np.sqrt(D)

USE_F32R = os.environ.get("ATTN_NO_F32R", "") == ""
USE_BF16 = os.environ.get("ATTN_BF16", "") != ""
ABL_HALFN = os.environ.get("ATTN_HALFN", "") != ""  # timing diagnostic only
ABL_NOEXP = os.environ.get("ATTN_NOEXP", "") != ""  # timing diagnostic only
ABL_NOMASK = os.environ.get("ATTN_NOMASK", "") != ""  # timing diagnostic only
BF16 = mybir.dt.bfloat16
# attention-operand dtype (k/q/exp(S)/v tiles)
AD = BF16 if USE_BF16 else F32


def _mm(ap):
    return ap.bitcast(F32R) if USE_F32R else ap


def _att(ap):
    # bf16 tiles are natively matmul-legal; fp32 tiles need the f32r bitcast
    return ap if USE_BF16 else _mm(ap)


def build_nc(t_len=T, repeat=1):
    assert t_len % SB == 0
    nsb = t_len // SB       # superblocks
    ntb = t_len // KB       # 128-blocks

    nc = bacc.Bacc("TRN2", target_bir_lowering=False, debug=False, num_devices=8)

    xb = nc.dram_tensor("xb", [t_len, E], F32, kind="ExternalInput")
    wqk = nc.dram_tensor("wqk", [E, 2 * CH], F32, kind="ExternalInput")
    wvp = nc.dram_tensor("wvp", [E, 256], F32, kind="ExternalInput")
    wo = nc.dram_tensor("wo", [CH, E], F32, kind="ExternalInput")
    bqk = nc.dram_tensor("bqk", [HL, 2, D], F32, kind="ExternalInput")
    bv = nc.dram_tensor("bv", [CH + HL], F32, kind="ExternalInput")
    cst = nc.dram_tensor("cst", [128, 192], F32, kind="ExternalInput")
    y = nc.dram_tensor("y", [t_len, E], F32, kind="ExternalOutput")

    xb, wqk, wvp, wo, bqk, bv, cst, y = (
        t.ap() for t in (xb, wqk, wvp, wo, bqk, bv, cst, y)
    )

    with tile.TileContext(nc) as tc:
        import contextlib

        ctx = contextlib.ExitStack()
        with ctx:
            ctx.enter_context(
                nc.allow_low_precision(reason="fp32r rounding of matmul operands")
            )
            const = ctx.enter_context(tc.tile_pool(name="const", bufs=1))
            persist = ctx.enter_context(tc.tile_pool(name="persist", bufs=1))
            xpool = ctx.enter_context(tc.tile_pool(name="xpool", bufs=2))
            xtpool = ctx.enter_context(tc.tile_pool(name="xtpool", bufs=2))
            qspool = ctx.enter_context(tc.tile_pool(name="qspool", bufs=4))
            ospool = ctx.enter_context(tc.tile_pool(name="ospool", bufs=2))
            ptpool = ctx.enter_context(tc.tile_pool(name="ptpool", bufs=3))
            rpool = ctx.enter_context(tc.tile_pool(name="rpool", bufs=2))
            ypool = ctx.enter_context(tc.tile_pool(name="ypool", bufs=2))
            psA = ctx.enter_context(tc.tile_pool(name="psA", bufs=2, space="PSUM"))
            psS = ctx.enter_context(tc.tile_pool(name="psS", bufs=2, space="PSUM"))
            psO = ctx.enter_context(tc.tile_pool(name="psO", bufs=2, space="PSUM"))

            # ---- constants / weights in SBUF ----
            # identity and the broadcast-ones row come in via DMA: walrus
            # requires f32r-typed producers for matmul operands and its ISA
            # rejects f32r memsets
            ident = const.tile([128, 128], F32)
            nc.sync.dma_start(out=_mm(ident), in_=_mm(cst[:, 0:128]))
            ones65 = const.tile([65, D], F32)
            nc.sync.dma_start(
                out=_mm(ones65[64:65, :]), in_=_mm(cst[64:65, 128 : 128 + D])
            )

            wqk_sb = const.tile([128, NEB, 2 * CH], F32)
            nc.sync.dma_start(
                out=_mm(wqk_sb), in_=_mm(wqk).rearrange("(n p) m -> p n m", p=128)
            )
            wv_sb = const.tile([128, NEB, 256], F32)
            nc.sync.dma_start(
                out=_mm(wv_sb), in_=_mm(wvp).rearrange("(n p) m -> p n m", p=128)
            )
            wo01_sb = const.tile([128, E], F32)
            nc.sync.dma_start(out=_mm(wo01_sb), in_=_mm(wo[0 : 2 * D, :]))
            wo2_sb = const.tile([D, E], F32)
            nc.sync.dma_start(out=_mm(wo2_sb), in_=_mm(wo[2 * D : CH, :]))
            bqk_sb = const.tile([D, HL, 2], F32)
            nc.sync.dma_start(out=bqk_sb, in_=bqk.rearrange("h q p -> p h q"))
            # k-bias copy living at partitions 64..127 (k rows of the packed
            # qk psum) so the staging add is partition-aligned
            bk64_sb = const.tile([128, HL], F32)
            nc.sync.dma_start(
                out=bk64_sb[D : 2 * D, :], in_=bqk[:, 1, :].rearrange("h p -> p h")
            )
            bv_bc = const.tile([128, CH + HL], F32)
            nc.sync.dma_start(
                out=bv_bc,
                in_=bass.AP(
                    tensor=bv.tensor, offset=bv.offset, ap=[[0, 128]] + list(bv.ap)
                ),
            )

            # persistent activations
            kT = [persist.tile([D, t_len], AD, name=f"kT{h}") for h in range(HL)]
            v_sb = persist.tile([128, ntb, HL, D + 1], AD)

            import contextlib as _cl

            loop_cm = tc.For_i(0, repeat, 1) if repeat > 1 else _cl.nullcontext()

            def phase3(cp, oS):
                # out-projection for t-blocks of superblock cp
                for tb in range(4):
                    tg = cp * 4 + tb
                    y_sb = ypool.tile([128, E], F32, tag="y_sb", name="y_sb")
                    oS01p, oS2p = oS
                    for half in range(2):
                        ps_y = psO.tile([128, 384], F32, tag="psO", name="ps_y")
                        nc.tensor.matmul(
                            ps_y,
                            lhsT=_mm(oS01p[:, tb * KB : (tb + 1) * KB]),
                            rhs=_mm(wo01_sb[:, half * 384 : (half + 1) * 384]),
                            start=True,
                            stop=False,
                        )
                        nc.tensor.matmul(
                            ps_y,
                            lhsT=_mm(oS2p[:, tb * KB : (tb + 1) * KB]),
                            rhs=_mm(wo2_sb[:, half * 384 : (half + 1) * 384]),
                            start=False,
                            stop=True,
                        )
                        nc.vector.tensor_copy(
                            out=y_sb[:, half * 384 : (half + 1) * 384], in_=ps_y
                        )
                    nc.sync.dma_start(
                        out=y[tg * KB : (tg + 1) * KB, :], in_=y_sb
                    )

            with loop_cm:
              for c in range(nsb):
                # ======== phase 1: x^T, q^T/k^T, v for tokens [c*SB, (c+1)*SB)
                x_nat = xpool.tile([128, 4, E], F32, tag="x")
                for tb in range(4):
                    t0 = c * SB + tb * KB
                    nc.sync.dma_start(
                        out=_mm(x_nat[:, tb, :]), in_=_mm(xb[t0 : t0 + KB, :])
                    )
                xT = xtpool.tile([128, NEB, SB], F32, tag="xT")
                for eb in range(NEB):
                    ps_t = psA.tile([128, SB], F32, tag="psA", name="ps_t")
                    for tb in range(4):
                        nc.tensor.transpose(
                            _mm(ps_t[:, tb * 128 : (tb + 1) * 128]),
                            _mm(x_nat[:, tb, eb * 128 : (eb + 1) * 128]),
                            _mm(ident),
                        )
                    nc.vector.tensor_copy(out=_mm(xT[:, eb, :]), in_=ps_t)
                qS = []
                for h in range(HL):
                    ps_qk = psA.tile([128, SB], F32, tag="psA", name="ps_qk")
                    for eb in range(NEB):
                        nc.tensor.matmul(
                            ps_qk,
                            lhsT=_mm(wqk_sb[:, eb, h * 128 : (h + 1) * 128]),
                            rhs=_mm(xT[:, eb, :]),
                            start=(eb == 0),
                            stop=(eb == NEB - 1),
                        )
                    q_h = qspool.tile([D, SB], AD, tag="qS", name="q_h")
                    nc.vector.tensor_scalar_add(
                        out=_att(q_h), in0=ps_qk[0:D, :], scalar1=bqk_sb[:, h, 0:1]
                    )
                    qS.append(q_h)
                    # k rows live at psum partitions 64..127. Lane engines
                    # cannot shift partitions, so stage at the same partitions
                    # (adding bias) and let an SBUF->SBUF DMA move them to
                    # partition base 0 in kT.
                    kst = qspool.tile([128, SB], AD, tag="kst", name="kst", bufs=2)
                    nc.vector.tensor_scalar_add(
                        out=_att(kst[D : 2 * D, :]),
                        in0=ps_qk[D : 2 * D, :],
                        scalar1=bk64_sb[D : 2 * D, h : h + 1],
                    )
                    nc.sync.dma_start(
                        out=_att(kT[h][:, c * SB : (c + 1) * SB]),
                        in_=_att(kst[D : 2 * D, :]),
                    )
                for tb in range(4):
                    ps_v = psA.tile([128, 256], F32, tag="psA", name="ps_v")
                    for eb in range(NEB):
                        nc.tensor.matmul(
                            ps_v,
                            lhsT=_mm(xT[:, eb, tb * 128 : (tb + 1) * 128]),
                            rhs=_mm(wv_sb[:, eb, :]),
                            start=(eb == 0),
                            stop=(eb == NEB - 1),
                        )
                    nc.vector.tensor_add(
                        out=_att(v_sb[:, c * 4 + tb, :, 0:D]),
                        in0=ps_v[:, 0:CH].rearrange("p (h d) -> p h d", h=HL),
                        in1=bv_bc[:, 0:CH].rearrange("p (h d) -> p h d", h=HL),
                    )
                    # ones column of v_aug: psum cols CH..CH+2 are x @ 0 = 0,
                    # plus the ones carried in the padded bias
                    nc.vector.tensor_add(
                        out=_att(v_sb[:, c * 4 + tb, :, D : D + 1]),
                        in0=ps_v[:, CH : CH + HL].rearrange(
                            "p (h o) -> p h o", o=1
                        ),
                        in1=bv_bc[:, CH : CH + HL].rearrange(
                            "p (h o) -> p h o", o=1
                        ),
                    )

                # phase 3 of previous superblock goes here: its inputs (oS)
                # are produced by a DVE chain that lags PE, so slot the
                # already-runnable phase-1 work of this chunk in front of it
                if c > 0:
                    phase3(c - 1, oS_prev)

                # ======== phase 2: attention superblock i=c, all local heads
                nj = 4 * c + 4
                oS01 = ospool.tile([128, SB], F32, tag="oS01", name="oS01")
                oS2 = ospool.tile([D, SB], F32, tag="oS2", name="oS2")
                oS_prev_local = (oS01, oS2)
                for h in range(HL):
                    ps_o = psO.tile([128, SB], F32, tag="psO", name="ps_o")
                    q_ap = _att(qS[h])

                    def q0_of(j):
                        # causal slice: key block j only sees queries
                        # >= j*KB - c*SB; keep the moving dim >= 256 so
                        # fp32r stays at full rate
                        if j < 4 * c:
                            return 0
                        return min((j - 4 * c) * KB, SB - 256)

                    def pv_step(j, pt_ap):
                        q0 = q0_of(j)
                        qe = (q0 + 256) if ABL_HALFN else SB
                        nc.tensor.matmul(
                            ps_o[0 : D + 1, q0:qe],
                            lhsT=_att(v_sb[:, j, h, :]),
                            rhs=_att(pt_ap[:, q0:qe]),
                            start=(j == 0),
                            stop=(j == nj - 1),
                        )

                    # process key blocks in pairs sharing one PSUM tile and
                    # (where widths match) a single wide exp on ACT, with a
                    # 2-block-deep software pipeline so PE stays ahead of ACT
                    pend = []
                    for jp in range(nj // 2):
                        j0, j1 = 2 * jp, 2 * jp + 1
                        q00, q01 = q0_of(j0), q0_of(j1)
                        ps_s2 = psS.tile([128, 2, SB], F32, tag="psS", name="ps_s2")
                        pt2 = ptpool.tile([128, 2, SB], AD, tag="pt", name="pt2")
                        for half, (j, q0) in enumerate(((j0, q00), (j1, q01))):
                            qe = (q0 + 256) if ABL_HALFN else SB
                            nc.tensor.matmul(
                                ps_s2[:, half, q0:qe],
                                lhsT=_att(kT[h][:, j * KB : (j + 1) * KB]),
                                rhs=q_ap[:, q0:qe],
                                start=True,
                                stop=True,
                            )
                        if ABL_NOEXP:
                            # keep the tile 'written' so Tile can release it
                            nc.vector.memset(pt2[:, :, 0:1], 0.5)
                        elif q00 == q01:
                            nc.scalar.activation(
                                out=_att(pt2[:, :, q00:]),
                                in_=ps_s2[:, :, q00:],
                                func=mybir.ActivationFunctionType.Exp,
                                scale=float(SCALE),
                            )
                        else:
                            for half, q0 in ((0, q00), (1, q01)):
                                nc.scalar.activation(
                                    out=_att(pt2[:, half, q0:]),
                                    in_=ps_s2[:, half, q0:],
                                    func=mybir.ActivationFunctionType.Exp,
                                    scale=float(SCALE),
                                )
                        for half, (j, q0) in enumerate(((j0, q00), (j1, q01))):
                            if j >= 4 * c and not (ABL_NOMASK or ABL_NOEXP):
                                nc.gpsimd.affine_select(
                                    out=_att(pt2[:, half, q0:]),
                                    in_=_att(pt2[:, half, q0:]),
                                    compare_op=mybir.AluOpType.is_ge,
                                    fill=0.0,
                                    base=c * SB - j * KB + q0,
                                    pattern=[[1, SB - q0]],
                                    channel_multiplier=-1,
                                )
                        pend.append((j0, pt2[:, 0, :]))
                        pend.append((j1, pt2[:, 1, :]))
                        while len(pend) > 4:
                            pv_step(*pend.pop(0))
                    for jq in pend:
                        pv_step(*jq)

                    recip = rpool.tile([65, SB], F32, tag="recip", name="recip")
                    nc.vector.reciprocal(_mm(recip[64:65, :]), ps_o[D : D + 1, :])
                    ps_b = psS.tile([128, SB], F32, tag="psS", name="ps_b")
                    nc.tensor.matmul(
                        ps_b[0:D, :],
                        lhsT=_mm(ones65[64:65, :]),
                        rhs=_mm(recip[64:65, :]),
                        start=True,
                        stop=True,
                    )
                    # walrus: a DVE op may read only ONE non-scalar PSUM input,
                    # so stage the broadcast row in SBUF before the multiply
                    rb = rpool.tile([D, SB], F32, tag="rbcast", name="rb")
                    nc.vector.tensor_copy(out=rb, in_=ps_b[0:D, :])
                    if h == 0:
                        o_dst = oS01[0:D, :]
                    elif h == 2:
                        o_dst = oS2[:, :]
                    else:
                        o_dst = ospool.tile(
                            [D, SB], F32, tag="o1tmp", name="o1tmp"
                        )
                    nc.vector.tensor_mul(_mm(o_dst), ps_o[0:D, :], rb)
                    if h == 1:
                        # stack h1 under h0 (partitions 64:128) via DMA, the
                        # only engine that can shift partitions
                        nc.sync.dma_start(out=_mm(oS01[D : 2 * D, :]), in_=_mm(o_dst))
                oS_prev = oS_prev_local
              phase3(nsb - 1, oS_prev)
    nc.compile()
    return nc


def make_in_maps(x, wq, bq, wk, bk, wv, bv, wo, bo, t_len=T):
    x = np.asarray(x, np.float32)
    in_maps = []
    for c in range(8):
        b, g = divmod(c, 4)
        hs = slice(g * CH, (g + 1) * CH)
        wqk_c = np.empty((E, 2 * CH), np.float32)
        bqk_c = np.empty((HL, 2, D), np.float32)
        for hl in range(HL):
            h = g * HL + hl
            wqk_c[:, hl * 128 : hl * 128 + D] = wq[:, h * D : (h + 1) * D]
            wqk_c[:, hl * 128 + D : (hl + 1) * 128] = wk[:, h * D : (h + 1) * D]
            bqk_c[hl, 0] = bq[h * D : (h + 1) * D]
            bqk_c[hl, 1] = bk[h * D : (h + 1) * D]
        wv_c = np.zeros((E, 256), np.float32)
        wv_c[:, :CH] = wv[:, hs]
        bv_c = np.ones(CH + HL, np.float32)
        bv_c[:CH] = np.asarray(bv, np.float32)[hs]
        cst = np.concatenate(
            [np.eye(128, dtype=np.float32), np.ones((128, 64), np.float32)], axis=1
        )
        in_maps.append(
            {
                "xb": np.ascontiguousarray(x[b, :t_len]),
                "wqk": wqk_c,
                "wvp": wv_c,
                "wo": np.ascontiguousarray(np.asarray(wo, np.float32)[hs]),
                "bqk": bqk_c,
                "bv": bv_c,
                "cst": cst,
            }
        )
    return in_maps


_NC_CACHE = {}


def get_nc(t_len=T):
    if t_len not in _NC_CACHE:
        _NC_CACHE[t_len] = build_nc(t_len)
    return _NC_CACHE[t_len]


def _build_sharded_nodonate(nc, n_cores=8):
    """Mirror bass2jax.run_bass_via_pjrt's multi-core path, minus donation,
    returning (jitted_fn, in_names, out_names, out_avals). Without donation a
    call can be repeated on device-resident arrays for timing. Safe here: the
    kernel writes every element of y."""
    import jax
    from jax.sharding import Mesh, PartitionSpec
    from jax.experimental.shard_map import shard_map

    from concourse import bass2jax
    from concourse.bass2jax import _bass_exec_p

    bass2jax.install_neuronx_cc_hook()
    part_name = nc.partition_id_tensor.name if nc.partition_id_tensor else None

    in_names, out_names, out_avals = [], [], []
    for alloc in nc.m.functions[0].allocations:
        if not isinstance(alloc, mybir.MemoryLocationSet):
            continue
        name = alloc.memorylocations[0].name
        if alloc.kind == "ExternalInput":
            if name != part_name:
                in_names.append(name)
        elif alloc.kind == "ExternalOutput":
            shape = tuple(alloc.tensor_shape)
            dtype = mybir.dt.np(alloc.dtype)
            out_names.append(name)
            out_avals.append(jax.core.ShapedArray(shape, dtype))
    n_params = len(in_names)
    all_names = in_names + out_names
    if part_name is not None:
        all_names = all_names + [part_name]

    def _body(*args):
        operands = list(args)
        if part_name is not None:
            operands.append(bass2jax.partition_id_tensor())
        outs = _bass_exec_p.bind(
            *operands,
            out_avals=tuple(out_avals),
            in_names=tuple(all_names),
            out_names=tuple(out_names),
            lowering_input_output_aliases=(),
            sim_require_finite=True,
            sim_require_nnan=True,
            nc=nc,
        )
        return tuple(outs)

    devices = jax.devices()[:n_cores]
    mesh = Mesh(np.asarray(devices), ("core",))
    n_out = len(out_names)
    sharded = jax.jit(
        shard_map(
            _body,
            mesh=mesh,
            in_specs=(PartitionSpec("core"),) * (n_params + n_out),
            out_specs=(PartitionSpec("core"),) * n_out,
            check_rep=False,
        ),
        keep_unused=True,
    )
    return sharded, in_names, out_names, out_avals


def run_timed(nc, in_maps, iters=20):
    """Execute on HW repeatedly with device-resident args; returns
    (per-core results, sorted per-call walls in seconds)."""
    import time

    import jax

    n_cores = len(in_maps)
    sharded, in_names, out_names, out_avals = _build_sharded_nodonate(nc, n_cores)
    concat_in = [
        np.concatenate([np.asarray(m[name]) for m in in_maps], axis=0)
        for name in in_names
    ]
    concat_zero = [
        np.zeros((n_cores * a.shape[0], *a.shape[1:]), a.dtype) for a in out_avals
    ]
    args = [jax.device_put(a) for a in concat_in + concat_zero]
    out = sharded(*args)  # compile + first run
    jax.block_until_ready(out)
    walls = []
    for _ in range(iters):
        t0 = time.perf_counter()
        out2 = sharded(*args)
        jax.block_until_ready(out2)
        walls.append(time.perf_counter() - t0)
    results = [
        {
            name: np.asarray(out[i]).reshape(n_cores, *out_avals[i].shape)[c]
            for i, name in enumerate(out_names)
        }
        for c in range(n_cores)
    ]
    return results, sorted(walls)


def baseline_rtt(iters=20):
    """Axon dispatch floor: same path with a trivial 8-core kernel."""
    nc = bacc.Bacc("TRN2", target_bir_lowering=False, debug=False, num_devices=8)
    a = nc.dram_tensor("a", [128, 128], F32, kind="ExternalInput")
    b = nc.dram_tensor("b", [128, 128], F32, kind="ExternalOutput")
    a, b = a.ap(), b.ap()
    with tile.TileContext(nc) as tc:
        with tc.tile_pool(name="p", bufs=1) as p:
            t = p.tile([128, 128], F32)
            nc.sync.dma_start(out=t, in_=a)
            nc.scalar.mul(out=t, in_=t, mul=2.0)
            nc.sync.dma_start(out=b, in_=t)
    nc.compile()
    in_maps = [{"a": np.zeros((128, 128), np.float32)} for _ in range(8)]
    _, walls = run_timed(nc, in_maps, iters=iters)
    return walls


def kernel(x, wq, bq, wk, bk, wv, bv, wo, bo, _trace=False, _trace_kwargs=None):
    nc = get_nc()
    in_maps = make_in_maps(x, wq, bq, wk, bk, wv, bv, wo, bo)
    res = run_bass_kernel_spmd(
        nc, in_maps, list(range(8)), trace=_trace, **(_trace_kwargs or {})
    )
    bo = np.asarray(bo, np.float32)
    out = np.empty((B, T, E), np.float32)
    for b in range(B):
        acc = res.results[b * 4]["y"].astype(np.float32).copy()
        for g in range(1, 4):
            acc += res.results[b * 4 + g]["y"]
        out[b] = acc + bo
    if _trace:
        return out, res
    return out
